# revision 1
# baseline (speedup 1.0000x reference)
"""MultiHeadAttention (B=2, S=2048, D=1024, H=16, HD=64) on 8 TRN2 cores.

Sharding: core i -> batch b = i//4, head-group g = i%4 (4 heads = 256 channels).
Each core computes its 4 heads end-to-end (QKV projection slices, attention,
out-projection partial) and writes a [2048, 1024] fp32 partial; host sums the
4 partials per batch and adds the constant bias terms (WV_b@Wout_w + Wout_b,
both of which commute through softmax-weighted averaging).

Schedule: the attention phase is ACT-bound (16.8M exps / 153.6 G/s ~= 110us),
so everything else hides under it. Input DMAs are seq-sliced so the first
score/exp fires ~13us in; all remaining projection and out-projection matmuls
are emitted as fine-grained PE "filler" units inside the attention site loop
(one site = exp + next-scores + attnV pair + fillers). Score matmul pairs
(K=64, lhsT at partition 0/64) row-quadrant-pack automatically and run
concurrently on the PE.

All matmul operands bf16 (fp8 tried and rejected: the attention output is
cancellation-suppressed to ~0.036 sigma_v, so fp8 quantization noise lands at
~3.6% relative instead of averaging down). PSUM f32.
"""

import math

import numpy as np
import ml_dtypes

B, S, D, H = 2, 2048, 1024, 16
HD = 64
P = 128
NQ = S // 512  # 4 q-blocks of 512
NK = S // 128  # 16 k-blocks of 128
BF16 = ml_dtypes.bfloat16

_CACHE = {}


def _build_nc():
    import concourse.bass as bass
    import concourse.mybir as mybir
    import concourse.tile as tile
    from concourse import bacc
    from concourse.bass import ds, ts

    f32 = mybir.dt.float32
    bf16 = mybir.dt.bfloat16

    nc = bacc.Bacc(None, target_bir_lowering=False, debug=False)

    xq_d = nc.dram_tensor("xq", [P, NQ, 8, 512], bf16, kind="ExternalInput")
    xk_d = nc.dram_tensor("xk", [P, NQ, 8, 512], bf16, kind="ExternalInput")
    xv_d = nc.dram_tensor("xv", [P, NK, 8, P], bf16, kind="ExternalInput")
    wq_d = nc.dram_tensor("wq", [P, 8, 2, P], bf16, kind="ExternalInput")
    wk_d = nc.dram_tensor("wk", [P, 8, 2, P], bf16, kind="ExternalInput")
    wv_d = nc.dram_tensor("wv", [P, 8, 256], bf16, kind="ExternalInput")
    wo_d = nc.dram_tensor("wo", [P, 2, 1024], bf16, kind="ExternalInput")
    bq_d = nc.dram_tensor("bq", [P, 2], f32, kind="ExternalInput")
    bk_d = nc.dram_tensor("bk", [P, 2], f32, kind="ExternalInput")
    out_d = nc.dram_tensor("out", [S, D], f32, kind="ExternalOutput")

    with tile.TileContext(nc) as tc:
        with (
            tc.tile_pool(name="persist", bufs=1) as pp,
            tc.tile_pool(name="ps", bufs=2, space="PSUM") as ps_pool,
            tc.tile_pool(name="ps_out", bufs=2, space="PSUM") as po_pool,
            tc.tile_pool(name="fillps", bufs=2, space="PSUM") as fill_pool,
            tc.tile_pool(name="attn", bufs=4) as attn_pool,
            tc.tile_pool(name="small", bufs=4) as small_pool,
        ):
            xq_sb = pp.tile([P, NQ, 8, 512], bf16, tag="xq_sb")
            xk_sb = pp.tile([P, NQ, 8, 512], bf16, tag="xk_sb")
            xv_sb = pp.tile([P, NK, 8, P], bf16, tag="xv_sb")
            wq_sb = pp.tile([P, 8, 2, P], bf16, tag="wq_sb")
            wk_sb = pp.tile([P, 8, 2, P], bf16, tag="wk_sb")
            wv_sb = pp.tile([P, 8, 256], bf16, tag="wv_sb")
            wo_sb = pp.tile([P, 2, 1024], bf16, tag="wo_sb")
            bq_sb = pp.tile([P, 2], f32, tag="bq_sb")
            bk_sb = pp.tile([P, 2], f32, tag="bk_sb")
            qhT = [
                pp.tile([P, S], bf16, tag=f"qhT{p}", name=f"qhT{p}")
                for p in range(2)
            ]
            khT = [
                pp.tile([P, S], bf16, tag=f"khT{p}", name=f"khT{p}")
                for p in range(2)
            ]
            vhx = pp.tile([P, NK, 260], bf16, tag="vhx")
            catT = [
                pp.tile([P, S], bf16, tag=f"catT{p}", name=f"catT{p}")
                for p in range(2)
            ]
            for h in range(4):
                nc.vector.memset(vhx[:, :, 65 * h + 64], 1.0)

            # DMA schedule: critical path first (wk+xk0 -> K-proj, wq+xq0 ->
            # Q-proj -> first scores ~13us in), then xv/xk/xq slices ordered
            # by filler deadline.
            nc.sync.dma_start(out=wk_sb[:], in_=wk_d[:])
            nc.sync.dma_start(out=bk_sb[:], in_=bk_d[:])
            nc.sync.dma_start(out=xk_sb[:, 0], in_=xk_d[:, 0])
            nc.sync.dma_start(out=wq_sb[:], in_=wq_d[:])
            nc.sync.dma_start(out=bq_sb[:], in_=bq_d[:])
            nc.sync.dma_start(out=xq_sb[:, 0], in_=xq_d[:, 0])
            nc.sync.dma_start(out=wv_sb[:], in_=wv_d[:])
            for kb in range(0, 6):
                nc.sync.dma_start(out=xv_sb[:, kb], in_=xv_d[:, kb])
            nc.sync.dma_start(out=xk_sb[:, 1], in_=xk_d[:, 1])
            for kb in range(6, 10):
                nc.sync.dma_start(out=xv_sb[:, kb], in_=xv_d[:, kb])
            nc.sync.dma_start(out=xk_sb[:, 2], in_=xk_d[:, 2])
            for kb in range(10, 13):
                nc.sync.dma_start(out=xv_sb[:, kb], in_=xv_d[:, kb])
            nc.sync.dma_start(out=xk_sb[:, 3], in_=xk_d[:, 3])
            for kb in range(13, NK):
                nc.sync.dma_start(out=xv_sb[:, kb], in_=xv_d[:, kb])
            for sblk in range(1, NQ):
                nc.sync.dma_start(out=xq_sb[:, sblk], in_=xq_d[:, sblk])
            nc.sync.dma_start(out=wo_sb[:], in_=wo_d[:])

            # ---- filler unit emitters (each unit ~2-4 matmuls of PE time)
            def qk_units(which, p, sb):
                x_sb, w_sb, b_sb, dst = (
                    (xq_sb, wq_sb, bq_sb, qhT)
                    if which == "q"
                    else (xk_sb, wk_sb, bk_sb, khT)
                )
                state = {}

                def unit(c0, p=p, sb=sb):
                    if c0 == 0:
                        state["acc"] = fill_pool.tile(
                            [P, 512], f32, name="proj_ps", tag="fill"
                        )
                    acc = state["acc"]
                    for c in (c0, c0 + 1):
                        nc.tensor.matmul(
                            acc[:],
                            w_sb[:, c, p],
                            x_sb[:, sb, c],
                            start=(c == 0),
                            stop=(c == 7),
                        )
                    if c0 == 6:
                        nc.vector.tensor_scalar_add(
                            dst[p][:, ts(sb, 512)], acc[:], b_sb[:, ds(p, 1)]
                        )

                return [lambda c0=c0: unit(c0) for c0 in (0, 2, 4, 6)]

            def v_units(sb):
                state = {}

                def unit(c0, sb=sb):
                    if c0 == 0:
                        state["acc"] = fill_pool.tile(
                            [P, 256], f32, name="vproj_ps", tag="fill"
                        )
                    acc = state["acc"]
                    for c in range(c0, c0 + 4):
                        nc.tensor.matmul(
                            acc[:],
                            xv_sb[:, sb, c],
                            wv_sb[:, c],
                            start=(c == 0),
                            stop=(c == 7),
                        )
                    if c0 == 4:
                        for h in range(4):
                            nc.vector.tensor_copy(
                                out=vhx[:, sb, ds(65 * h, 64)],
                                in_=acc[:, ds(64 * h, 64)],
                            )

                return [lambda: unit(0), lambda: unit(4)]

            def o_unit(sb, nh):
                def unit(sb=sb, nh=nh):
                    acc = fill_pool.tile(
                        [P, 512], f32, name="oproj_ps", tag="fill"
                    )
                    for cc in range(2):
                        nc.tensor.matmul(
                            acc[:],
                            catT[cc][:, ts(sb, P)],
                            wo_sb[:, cc, ts(nh, 512)],
                            start=(cc == 0),
                            stop=(cc == 1),
                        )
                    osb = small_pool.tile([P, 512], f32, name="oevict")
                    nc.vector.tensor_copy(out=osb[:], in_=acc[:])
                    # output DMAs ride the (idle) gpsimd queue so the tail's
                    # catT DMA on sync isn't stuck behind ~2MB of output
                    nc.gpsimd.dma_start(
                        out=out_d[ts(sb, P), ts(nh, 512)], in_=osb[:]
                    )

                return unit

            def inline(units):
                for u in units:
                    u()

            def emit_scores(p, qb, kb):
                sc = ps_pool.tile([P, 1024], f32, name="scores_ps", tag="ps")
                for ab in range(2):
                    nc.tensor.matmul(
                        sc[:, ds(512 * ab, 512)],
                        khT[p][ds(64 * ab, 64), ts(kb, P)],
                        qhT[p][ds(64 * ab, 64), ts(qb, 512)],
                        start=True,
                        stop=True,
                    )
                return sc

            def emit_normalize(p, qb, oAB):
                # reciprocal_approx_fast (custom DVE) executes as garbage on
                # this HW stack; use the bit-exact microcoded reciprocal,
                # batched over both head-pairs' sum rows in one [2,512] call.
                o_sbs = []
                for ab in range(2):
                    o_sb = small_pool.tile([65, 512], f32, name="o_sb")
                    nc.vector.tensor_copy(out=o_sb[:], in_=oAB[ab][:])
                    o_sbs.append(o_sb)
                stage = small_pool.tile([2, 512], f32, name="sumrows")
                for ab in range(2):
                    nc.sync.dma_start(
                        out=stage[ds(ab, 1), :], in_=o_sbs[ab][ds(64, 1), :]
                    )
                rstage = small_pool.tile([2, 512], f32, name="rsums")
                nc.vector.reciprocal(rstage[:], stage[:])
                # partition_broadcast reads absolute partition 0 on HW:
                # row 0 of rstage is already there; DMA-shift row 1.
                r1 = small_pool.tile([1, 512], f32, name="r1")
                nc.sync.dma_start(out=r1[:], in_=rstage[ds(1, 1), :])
                for ab in range(2):
                    o_sb = o_sbs[ab]
                    src = rstage[ds(0, 1), :] if ab == 0 else r1[:]
                    bcs = small_pool.tile([64, 512], f32, name="bcast_sb")
                    nc.gpsimd.partition_broadcast(bcs[:], src, channels=64)
                    if ab == 0:
                        nc.vector.tensor_tensor(
                            out=catT[p][ds(0, 64), ts(qb, 512)],
                            in0=o_sb[ds(0, 64), :],
                            in1=bcs[:],
                            op=mybir.AluOpType.mult,
                        )
                    else:
                        tmp = small_pool.tile([64, 512], bf16, name="normB")
                        nc.vector.tensor_tensor(
                            out=tmp[:],
                            in0=o_sb[ds(0, 64), :],
                            in1=bcs[:],
                            op=mybir.AluOpType.mult,
                        )
                        nc.sync.dma_start(
                            out=catT[p][ds(64, 64), ts(qb, 512)], in_=tmp[:]
                        )

            # ---- pre-attention critical path: K sb0, Q sb0, first scores,
            # then vproj sb0-2 (first exp must not wait behind v-units)
            inline(qk_units("k", 0, 0))
            inline(qk_units("q", 0, 0))
            sc_next = emit_scores(0, 0, 0)
            for sb in range(3):
                inline(v_units(sb))

            # ---- filler queue for qb0 (arrival-ordered: xv0-5 land ~14us,
            # xk1 ~18us, xk2 ~22us, xk3 ~26us, xq1 ~30us)
            fillq = []
            fillq += v_units(3) + v_units(4) + v_units(5)
            fillq += qk_units("k", 0, 1)
            fillq += v_units(6) + v_units(7) + v_units(8) + v_units(9)
            fillq += qk_units("k", 0, 2)
            fillq += v_units(10) + v_units(11) + v_units(12)
            fillq += qk_units("k", 0, 3)
            fillq += v_units(13) + v_units(14) + v_units(15)
            fillq += qk_units("q", 0, 1)
            # per-site pull counts for qb0 (sites 12-15 let ACT catch up)
            pulls_qb0 = [0, 2, 4, 4, 4, 4, 4, 4, 4, 4, 4, 4, 0, 0, 0, 0]
            # fillers for later sites (1 pull/site)
            late = []
            late += qk_units("q", 0, 2) + qk_units("q", 0, 3)
            for sb in range(NQ):
                late += qk_units("k", 1, sb)
            for sb in range(NQ):
                late += qk_units("q", 1, sb)

            def pull(n):
                for _ in range(n):
                    if fillq:
                        u = fillq.pop(0)
                        if u is not None:
                            u()

            # ---- attention site loop
            for p in range(2):
                for qb in range(NQ):
                    oAB = [
                        po_pool.tile(
                            [65, 512], f32, name=f"outT{ab}", tag="outT"
                        )
                        for ab in range(2)
                    ]
                    for kb in range(NK):
                        sc = sc_next
                        at = attn_pool.tile([P, 1024], bf16, name="attnT")
                        nc.scalar.activation(
                            at[:], sc[:], mybir.ActivationFunctionType.Exp
                        )
                        qb0 = (p, qb) == (0, 0)
                        if not qb0:
                            # ACT-bound phase: feed next scores immediately
                            if kb + 1 < NK:
                                sc_next = emit_scores(p, qb, kb + 1)
                            elif (p, qb) != (1, NQ - 1):
                                np_, nqb = (
                                    (p, qb + 1) if qb + 1 < NQ else (p + 1, 0)
                                )
                                sc_next = emit_scores(np_, nqb, 0)
                        for ab in range(2):
                            nc.tensor.matmul(
                                oAB[ab][:],
                                vhx[:, kb, ds(65 * (2 * p + ab), 65)],
                                at[:, ds(512 * ab, 512)],
                                start=(kb == 0),
                                stop=(kb == NK - 1),
                            )
                        if qb0:
                            # PE-bound phase: pulls precede next scores so
                            # khT/qhT producers sit before their consumer in
                            # the in-order PE queue
                            pull(pulls_qb0[kb])
                            sc_next = emit_scores(
                                *((0, 0, kb + 1) if kb + 1 < NK else (0, 1, 0))
                            )
                        else:
                            pull(1)
                    if (p, qb) == (0, 0):
                        fillq.extend(late)
                    emit_normalize(p, qb, oAB)
                    if p == 1:
                        # 1 o-unit per 2 sites keeps per-site PE load under
                        # the ACT cadence
                        for sb in range(4 * qb, 4 * qb + 4):
                            for nh in range(2):
                                fillq.append(o_unit(sb, nh))
                                fillq.append(None)
            # drain remaining fillers (out-projection of the last q-block)
            while fillq:
                u = fillq.pop(0)
                if u is not None:
                    u()

    nc.compile()
    return nc


def _prep_inputs(q, k, v, WQ_w, WQ_b, WK_w, WK_b, WV_w, WV_b, Wout_w, Wout_b):
    scale = 1.0 / math.sqrt(HD)

    def chunk_qk(x):  # [S, D] -> [P, NQ, 8, 512]
        return np.ascontiguousarray(
            x.T.reshape(8, P, NQ, 512).transpose(1, 2, 0, 3)
        ).astype(BF16)

    def chunk_v(x):  # [S, D] -> [P, NK, 8, 128]
        return np.ascontiguousarray(
            x.T.reshape(8, P, NK, P).transpose(1, 2, 0, 3)
        ).astype(BF16)

    xqs = [chunk_qk(q[b]) for b in range(B)]
    xks = [chunk_qk(k[b]) for b in range(B)]
    xvs = [chunk_v(v[b]) for b in range(B)]

    in_maps = []
    for i in range(8):
        b, g = divmod(i, 4)
        sl = slice(256 * g, 256 * (g + 1))
        wq = (WQ_w[:, sl] * scale).reshape(8, P, 2, P).transpose(1, 0, 2, 3)
        wk = WK_w[:, sl].reshape(8, P, 2, P).transpose(1, 0, 2, 3)
        wv = WV_w[:, sl].reshape(8, P, 256).transpose(1, 0, 2)
        wo = Wout_w[sl, :].reshape(2, P, 1024).transpose(1, 0, 2)
        bq = (WQ_b[sl] * scale).reshape(2, P).T
        bk = WK_b[sl].reshape(2, P).T
        in_maps.append(
            {
                "xq": xqs[b],
                "xk": xks[b],
                "xv": xvs[b],
                "wq": np.ascontiguousarray(wq).astype(BF16),
                "wk": np.ascontiguousarray(wk).astype(BF16),
                "wv": np.ascontiguousarray(wv).astype(BF16),
                "wo": np.ascontiguousarray(wo).astype(BF16),
                "bq": np.ascontiguousarray(bq, dtype=np.float32),
                "bk": np.ascontiguousarray(bk, dtype=np.float32),
            }
        )
    return in_maps


def run(trace=False, **inputs):
    from concourse.bass_utils import run_bass_kernel_spmd

    if "nc" not in _CACHE:
        _CACHE["nc"] = _build_nc()
    nc = _CACHE["nc"]

    in_maps = _prep_inputs(**inputs)
    res = run_bass_kernel_spmd(nc, in_maps, list(range(8)), trace=trace)

    const = (
        inputs["WV_b"].astype(np.float32) @ inputs["Wout_w"].astype(np.float32)
        + inputs["Wout_b"].astype(np.float32)
    )
    out = np.zeros((B, S, D), dtype=np.float32)
    for i in range(8):
        b = i // 4
        out[b] += res.results[i]["out"]
    out += const[None, None, :]
    return out, res


def kernel(**inputs):
    out, _ = run(trace=False, **inputs)
    return out

